# revision 24
# baseline (speedup 1.0000x reference)
"""Trainium2 Bass kernel for BioNet message-passing recurrence.

Computes 50 steps of  X <- mml(W @ X + X_bias)  with W (8192x8192 f32,
masked) and X (8192x32), returning X.T (32, 8192).

Strategy (8 NeuronCores, tensor-parallel over W rows):
  - Each core holds rows [1024c, 1024c+1024) of W, stored transposed in
    SBUF as bf16 (16.8 MB/core) for the whole kernel -> no per-step HBM
    traffic for W.
  - Per step, each core computes its 1024 rows of W @ X as
    out^T = X^T @ W_shard^T on the PE with X (128,32) tiles stationary
    and W streaming, 4-way column-tiled (4 concurrent 32-wide stationary
    tiles, one per K-subset) for ~4x PE throughput at batch=32.
  - The 4 column-group partials land on partition groups 32j..32j+32 of
    PSUM; a second small PE pass multiplies by a selector matrix
    S[p,b] = (p%32==b) which fuses the 4-way reduction with the
    (batch,node) -> (node,batch) transpose.
  - Bias + Michaelis-Menten activation on DVE; the activated (1024,32)
    bf16 chunk is AllGathered across the 8 cores for the next step.
  - The output is split in two 512-node halves with two staggered
    AllGathers: the next step's matmuls are reordered so the K-tiles
    fed by AllGather A run first, hiding AllGather B under compute.
"""

import os
import sys
import types

sys.path.insert(0, "/opt/trn_rl_repo")

import numpy as np
import ml_dtypes

import concourse.bass as bass
import concourse.mybir as mybir
import concourse.tile as tile
from concourse import bacc
import concourse.bass_utils as bass_utils
from concourse.bass import ts
from concourse.bass_utils import run_bass_kernel_spmd

N_NODES = 8192
N_CORES = 8
BATCH = 32
MAX_STEPS = 50
LEAK = 0.01
LOCAL = N_NODES // N_CORES          # 1024 rows per core
K_TILES = N_NODES // 128            # 64
LOCAL_TILES = LOCAL // 128          # 8
CHUNK_F = LOCAL_TILES * BATCH       # 256 free elems per activated chunk
HALF_F = CHUNK_F // 2               # 128

LAST_RESULTS = None  # BassKernelResults of the most recent run (for test.py)


def setup_tracing():
    """Register the axon NTFF profile hook; the container's antenv is a stub."""
    try:
        import antenv
        if "antenv.axon_hooks" not in sys.modules:
            mod = types.ModuleType("antenv.axon_hooks")
            mod._hook = None
            mod.set_axon_ntff_profile_hook = lambda h: setattr(mod, "_hook", h)
            mod.get_axon_ntff_profile_hook = lambda: mod._hook
            sys.modules["antenv.axon_hooks"] = mod
            antenv.axon_hooks = mod
            from trn_agent_boot.trn_boot import _ntff_profile_via_ctypes
            mod.set_axon_ntff_profile_hook(
                _ntff_profile_via_ctypes("/opt/axon/libaxon_pjrt.so")
            )
        bass_utils.upload_artifacts = lambda tmpdir: f"local://{tmpdir}"
    except Exception:
        pass


def build_nc():
    nc = bacc.Bacc(None, target_bir_lowering=False, num_devices=N_CORES)
    f32 = mybir.dt.float32
    bf16 = mybir.dt.bfloat16

    # Per-core inputs (shapes identical on every core; contents sharded).
    wt = nc.dram_tensor("wt", [N_NODES, LOCAL], bf16, kind="ExternalInput")
    xb = nc.dram_tensor("xb", [128, CHUNK_F], f32, kind="ExternalInput")
    s_in = nc.dram_tensor("s_in", [128, BATCH], bf16, kind="ExternalInput")
    out = nc.dram_tensor("out", [128, CHUNK_F], f32, kind="ExternalOutput")

    with tile.TileContext(nc) as tc:
        with (
            tc.tile_pool(name="persist", bufs=1) as persist,
            tc.tile_pool(name="ys", bufs=2) as ys_pool,
            tc.tile_pool(name="chain", bufs=2) as chain,
            tc.tile_pool(name="stage", bufs=3) as stage_pool,
            tc.tile_pool(name="psum", bufs=2, space="PSUM") as psum_pool,
            tc.tile_pool(name="psumt", bufs=2, space="PSUM") as psumt_pool,
            tc.tile_pool(name="dram", bufs=2, space="DRAM") as dram,
        ):
            # ---- persistent SBUF tensors -------------------------------
            wt_sb = persist.tile([128, K_TILES, LOCAL], bf16)      # 128 KB/part
            wt_v = wt.rearrange("(t p) n -> p t n", p=128)
            nc.sync.dma_start(
                out=wt_sb[:, 0 : K_TILES // 2, :], in_=wt_v[:, 0 : K_TILES // 2, :]
            )
            nc.scalar.dma_start(
                out=wt_sb[:, K_TILES // 2 :, :], in_=wt_v[:, K_TILES // 2 :, :]
            )
            xb_sb = persist.tile([128, CHUNK_F], f32)
            nc.sync.dma_start(out=xb_sb, in_=xb[:])
            s_sb = persist.tile([128, BATCH], bf16)
            nc.sync.dma_start(out=s_sb, in_=s_in[:])
            x_sb = persist.tile([128, K_TILES * BATCH], bf16)      # gathered state

            def activation(z_src, to_bf, also_f32=None, width=CHUNK_F):
                """to_bf[:] = mml(z_src) in bf16; optionally also f32 copy.

                mml(z) = max(leak*z, min(z, 1 - 0.25/max(z, 0.5)))
                (exact for |z| < ~99, which holds here).
                """
                m_t = chain.tile([128, width], f32, tag="m", name="m_t")
                nc.vector.tensor_scalar_max(m_t, z_src, 0.5)
                r_t = chain.tile([128, width], f32, tag="r", name="r_t")
                nc.vector.reciprocal_approx_fast(out=r_t, in_=m_t)
                s_t = chain.tile([128, width], f32, tag="s", name="s_t")
                nc.vector.tensor_scalar(
                    s_t, r_t, -0.25, 1.0,
                    mybir.AluOpType.mult, mybir.AluOpType.add,
                )
                t_t = chain.tile([128, width], f32, tag="t", name="t_t")
                nc.vector.tensor_tensor(t_t, z_src, s_t, mybir.AluOpType.min)
                # out = (z * leak) max t
                nc.vector.scalar_tensor_tensor(
                    to_bf, z_src, LEAK, t_t,
                    mybir.AluOpType.mult, mybir.AluOpType.max,
                )
                if also_f32 is not None:
                    nc.vector.scalar_tensor_tensor(
                        also_f32, z_src, LEAK, t_t,
                        mybir.AluOpType.mult, mybir.AluOpType.max,
                    )

            def tail_half(psum_hv, v, out_f32):
                """Reduce+transpose (S-matrix PE pass), bias+activation for
                output half v; returns the staged bf16 (128, HALF_F) tile."""
                ysb = ys_pool.tile([128, 512], bf16, tag="ysb", name="ysb")
                nc.vector.tensor_copy(ysb, psum_hv)
                psum_t = psumt_pool.tile(
                    [128, HALF_F], mybir.dt.float32, tag="pt", name="psum_t"
                )
                for tt_ in range(4):
                    nc.tensor.matmul(
                        psum_t[:, ts(tt_, BATCH)],
                        ysb[:, ts(tt_, 128)],
                        s_sb,
                        start=True,
                        stop=True,
                    )
                hs = ts(v, HALF_F)
                z_t = chain.tile([128, HALF_F], mybir.dt.float32,
                                 tag="z", name="z_t")
                nc.vector.tensor_tensor(
                    z_t, psum_t, xb_sb[:, hs], mybir.AluOpType.add
                )
                stage_v = stage_pool.tile(
                    [128, HALF_F], bf16, tag=f"st{v}", name=f"stage{v}"
                )
                activation(
                    z_t,
                    stage_v,
                    also_f32=None if out_f32 is None else out_f32[:, hs],
                    width=HALF_F,
                )
                return stage_v

            def broadcast(stage_a, stage_b):
                """AllGather both staged halves into x_sb."""
                ag_in = dram.tile([128, CHUNK_F], bf16, tag="agi", name="ag_in")
                nc.sync.dma_start(out=ag_in[:, 0:HALF_F], in_=stage_a)
                nc.scalar.dma_start(out=ag_in[:, HALF_F:CHUNK_F], in_=stage_b)
                ag_out = dram.tile(
                    [128 * N_CORES, CHUNK_F], bf16, addr_space="Shared",
                    tag="ago", name="ag_out",
                )
                nc.gpsimd.collective_compute(
                    "AllGather",
                    mybir.AluOpType.bypass,
                    replica_groups=[list(range(N_CORES))],
                    ins=[ag_in.opt()],
                    outs=[ag_out.opt()],
                )
                # per-source-core chunk DMAs (two HWDGE engines) so the next
                # step's first quads start before the whole state has landed
                for c in range(N_CORES):
                    eng = nc.sync if c % 2 == 0 else nc.scalar
                    eng.dma_start(
                        out=x_sb[:, CHUNK_F * c : CHUNK_F * (c + 1)],
                        in_=ag_out[128 * c : 128 * (c + 1), :],
                    )

            # PE warm-keeping: DVE scratch copies act as coarse timers that
            # pace small dummy-matmul bursts through the AllGather window so
            # HAM never sees a >3.4us idle gap on the PE array.
            pace_cols = int(os.environ.get("PACE_COLS", "4096"))
            n_bursts = int(os.environ.get("WARM_BURSTS", "4"))
            warm_per = int(os.environ.get("WARM_PER", "3"))
            pw_a = persist.tile([128, pace_cols], f32, name="pw_a")
            pw_b = persist.tile([128, pace_cols], f32, name="pw_b")
            nc.vector.memset(pw_a, 0.0)
            nc.vector.memset(pw_b, 0.0)

            def pe_warm():
                psum_w = psumt_pool.tile(
                    [128, 512], mybir.dt.float32, tag="pw", name="psum_w",
                    bufs=1,
                )

                def burst(dep):
                    for _ in range(warm_per):
                        wmm = nc.tensor.matmul(
                            psum_w[0:BATCH, :], s_sb, wt_sb[:, 0, 0:512],
                            start=True, stop=True,
                        )
                        if dep is not None:
                            bass._add_dep_helper(
                                wmm.ins, dep.ins, True, "pace warm mm"
                            )

                burst(None)
                for i in range(n_bursts):
                    src, dst = (pw_a, pw_b) if i % 2 == 0 else (pw_b, pw_a)
                    cp = nc.vector.tensor_copy(dst, src)
                    burst(cp)

            # ---- step 1: X1 = mml(X_bias) ------------------------------
            stage_halves = []
            for v in range(2):
                stage_v = stage_pool.tile(
                    [128, HALF_F], bf16, tag=f"st{v}", name=f"stage{v}"
                )
                activation(xb_sb[:, ts(v, HALF_F)], stage_v, width=HALF_F)
                stage_halves.append(stage_v)
            broadcast(*stage_halves)

            # ---- steps 2..50: X <- mml(W @ X + X_bias) -----------------
            n_quads = K_TILES // 4  # 16
            for step in range(MAX_STEPS - 1):
                last = step == MAX_STEPS - 2
                out_f32 = None
                if last:
                    out_f32 = stage_pool.tile(
                        [128, CHUNK_F], mybir.dt.float32, tag="of", name="out_f32"
                    )
                # main matmul, h (output half) outer so half 0's full tail
                # overlaps half 1's matmuls; 4-way column-tiled over K
                psum_h = [
                    psum_pool.tile(
                        [128, 512], mybir.dt.float32, tag="pa", name="psum_a"
                    ),
                    psum_pool.tile(
                        [128, 512], mybir.dt.float32, tag="pb", name="psum_b"
                    ),
                ]

                def mm_quads(h, quads):
                    for q in quads:
                        for j in range(4):
                            k = 4 * q + j
                            nc.tensor.matmul(
                                psum_h[h][32 * j : 32 * (j + 1), :],
                                x_sb[:, ts(k, BATCH)],
                                wt_sb[:, k, ts(h, 512)],
                                start=(q == 0),
                                stop=(q == n_quads - 1),
                                tile_position=(0, 32 * j),
                            )

                mm_quads(0, range(n_quads))
                mm_quads(1, range(n_quads // 2))
                stage_a = tail_half(psum_h[0], 0, out_f32)  # S-pass lands here
                mm_quads(1, range(n_quads // 2, n_quads))
                stage_b = tail_half(psum_h[1], 1, out_f32)
                if last:
                    nc.sync.dma_start(out=out[:], in_=out_f32)
                else:
                    broadcast(stage_a, stage_b)
                    pe_warm()

    nc.compile()
    return nc


def _prepare_in_maps(X_full, weights, bias, edge_mask):
    W = np.where(edge_mask, weights, 0.0).astype(np.float32)
    Xb = X_full.astype(np.float32).T + bias.astype(np.float32)  # (n, B)
    S = np.zeros((128, BATCH), np.float32)
    S[np.arange(128), np.arange(128) % BATCH] = 1.0
    S = S.astype(ml_dtypes.bfloat16)
    in_maps = []
    for c in range(N_CORES):
        rows = slice(LOCAL * c, LOCAL * (c + 1))
        wt_c = np.ascontiguousarray(W[rows, :].T).astype(ml_dtypes.bfloat16)
        xb_c = (
            Xb[rows]                       # (1024, 32)
            .reshape(LOCAL_TILES, 128, BATCH)
            .transpose(1, 0, 2)
            .reshape(128, CHUNK_F)
            .copy()
        )
        in_maps.append({"wt": wt_c, "xb": xb_c, "s_in": S})
    return in_maps


def _reassemble(results):
    out = np.empty((BATCH, N_NODES), np.float32)
    for c in range(N_CORES):
        oc = np.asarray(results[c]["out"])  # (128, 256)
        chunk = (
            oc.reshape(128, LOCAL_TILES, BATCH)
            .transpose(1, 0, 2)
            .reshape(LOCAL, BATCH)
        )
        out[:, LOCAL * c : LOCAL * (c + 1)] = chunk.T
    return out


def kernel(X_full, weights, bias, edge_mask):
    global LAST_RESULTS
    setup_tracing()
    in_maps = _prepare_in_maps(X_full, weights, bias, edge_mask)
    nc = build_nc()
    res = run_bass_kernel_spmd(nc, in_maps, core_ids=list(range(N_CORES)))
    LAST_RESULTS = res
    return _reassemble(res.results)


if __name__ == "__main__":
    # quick self-run with random data
    rng = np.random.default_rng(0)
    X_full = rng.random((BATCH, N_NODES), np.float32)
    weights = rng.standard_normal((N_NODES, N_NODES), np.float32)
    bias = 0.001 * np.ones((N_NODES, 1), np.float32)
    edge_mask = rng.random((N_NODES, N_NODES)) < 0.002
    out = kernel(X_full, weights, bias, edge_mask)
    print("out", out.shape, out.dtype, out[:2, :4])


# revision 25
# speedup vs baseline: 1.1250x; 1.1250x over previous
"""Trainium2 Bass kernel for BioNet message-passing recurrence.

Computes 50 steps of  X <- mml(W @ X + X_bias)  with W (8192x8192 f32,
masked) and X (8192x32), returning X.T (32, 8192).

Strategy (8 NeuronCores, tensor-parallel over W rows):
  - Each core holds rows [1024c, 1024c+1024) of W, stored transposed in
    SBUF as bf16 (16.8 MB/core) for the whole kernel -> no per-step HBM
    traffic for W.
  - Per step, each core computes its 1024 rows of W @ X as
    out^T = X^T @ W_shard^T on the PE with X (128,32) tiles stationary
    and W streaming, 4-way column-tiled (4 concurrent 32-wide stationary
    tiles, one per K-subset) for ~4x PE throughput at batch=32.
  - The 4 column-group partials land on partition groups 32j..32j+32 of
    PSUM; a second small PE pass multiplies by a selector matrix
    S[p,b] = (p%32==b) which fuses the 4-way reduction with the
    (batch,node) -> (node,batch) transpose.
  - Bias + Michaelis-Menten activation on DVE; the activated (1024,32)
    bf16 chunk is AllGathered across the 8 cores for the next step.
  - The output is split in two 512-node halves with two staggered
    AllGathers: the next step's matmuls are reordered so the K-tiles
    fed by AllGather A run first, hiding AllGather B under compute.
"""

import os
import sys
import types

sys.path.insert(0, "/opt/trn_rl_repo")

import numpy as np
import ml_dtypes

import concourse.bass as bass
import concourse.mybir as mybir
import concourse.tile as tile
from concourse import bacc
import concourse.bass_utils as bass_utils
from concourse.bass import ts
from concourse.bass_utils import run_bass_kernel_spmd

N_NODES = 8192
N_CORES = 8
BATCH = 32
MAX_STEPS = 50
LEAK = 0.01
LOCAL = N_NODES // N_CORES          # 1024 rows per core
K_TILES = N_NODES // 128            # 64
LOCAL_TILES = LOCAL // 128          # 8
CHUNK_F = LOCAL_TILES * BATCH       # 256 free elems per activated chunk
HALF_F = CHUNK_F // 2               # 128

LAST_RESULTS = None  # BassKernelResults of the most recent run (for test.py)


def setup_tracing():
    """Register the axon NTFF profile hook; the container's antenv is a stub."""
    try:
        import antenv
        if "antenv.axon_hooks" not in sys.modules:
            mod = types.ModuleType("antenv.axon_hooks")
            mod._hook = None
            mod.set_axon_ntff_profile_hook = lambda h: setattr(mod, "_hook", h)
            mod.get_axon_ntff_profile_hook = lambda: mod._hook
            sys.modules["antenv.axon_hooks"] = mod
            antenv.axon_hooks = mod
            from trn_agent_boot.trn_boot import _ntff_profile_via_ctypes
            mod.set_axon_ntff_profile_hook(
                _ntff_profile_via_ctypes("/opt/axon/libaxon_pjrt.so")
            )
        bass_utils.upload_artifacts = lambda tmpdir: f"local://{tmpdir}"
    except Exception:
        pass


def build_nc():
    nc = bacc.Bacc(None, target_bir_lowering=False, num_devices=N_CORES)
    f32 = mybir.dt.float32
    bf16 = mybir.dt.bfloat16

    # Per-core inputs (shapes identical on every core; contents sharded).
    wt = nc.dram_tensor("wt", [N_NODES, LOCAL], bf16, kind="ExternalInput")
    xb = nc.dram_tensor("xb", [128, CHUNK_F], f32, kind="ExternalInput")
    s_in = nc.dram_tensor("s_in", [128, BATCH], bf16, kind="ExternalInput")
    out = nc.dram_tensor("out", [128, CHUNK_F], f32, kind="ExternalOutput")

    with tile.TileContext(nc) as tc:
        with (
            tc.tile_pool(name="persist", bufs=1) as persist,
            tc.tile_pool(name="ys", bufs=2) as ys_pool,
            tc.tile_pool(name="chain", bufs=2) as chain,
            tc.tile_pool(name="stage", bufs=3) as stage_pool,
            tc.tile_pool(name="psum", bufs=2, space="PSUM") as psum_pool,
            tc.tile_pool(name="psumt", bufs=2, space="PSUM") as psumt_pool,
            tc.tile_pool(name="dram", bufs=2, space="DRAM") as dram,
        ):
            # ---- persistent SBUF tensors -------------------------------
            wt_sb = persist.tile([128, K_TILES, LOCAL], bf16)      # 128 KB/part
            wt_v = wt.rearrange("(t p) n -> p t n", p=128)
            nc.sync.dma_start(
                out=wt_sb[:, 0 : K_TILES // 2, :], in_=wt_v[:, 0 : K_TILES // 2, :]
            )
            nc.scalar.dma_start(
                out=wt_sb[:, K_TILES // 2 :, :], in_=wt_v[:, K_TILES // 2 :, :]
            )
            xb_sb = persist.tile([128, CHUNK_F], f32)
            nc.sync.dma_start(out=xb_sb, in_=xb[:])
            s_sb = persist.tile([128, BATCH], bf16)
            nc.sync.dma_start(out=s_sb, in_=s_in[:])
            x_sb = persist.tile([128, K_TILES * BATCH], bf16)      # gathered state

            def activation(z_src, to_bf, also_f32=None, width=CHUNK_F):
                """to_bf[:] = mml(z_src) in bf16; optionally also f32 copy.

                mml(z) = max(leak*z, min(z, 1 - 0.25/max(z, 0.5)))
                (exact for |z| < ~99, which holds here).
                """
                m_t = chain.tile([128, width], f32, tag="m", name="m_t")
                nc.vector.tensor_scalar_max(m_t, z_src, 0.5)
                r_t = chain.tile([128, width], f32, tag="r", name="r_t")
                nc.vector.reciprocal_approx_fast(out=r_t, in_=m_t)
                s_t = chain.tile([128, width], f32, tag="s", name="s_t")
                nc.vector.tensor_scalar(
                    s_t, r_t, -0.25, 1.0,
                    mybir.AluOpType.mult, mybir.AluOpType.add,
                )
                t_t = chain.tile([128, width], f32, tag="t", name="t_t")
                nc.vector.tensor_tensor(t_t, z_src, s_t, mybir.AluOpType.min)
                # out = (z * leak) max t
                nc.vector.scalar_tensor_tensor(
                    to_bf, z_src, LEAK, t_t,
                    mybir.AluOpType.mult, mybir.AluOpType.max,
                )
                if also_f32 is not None:
                    nc.vector.scalar_tensor_tensor(
                        also_f32, z_src, LEAK, t_t,
                        mybir.AluOpType.mult, mybir.AluOpType.max,
                    )

            def tail_half(psum_hv, v, out_f32):
                """Reduce+transpose (S-matrix PE pass), bias+activation for
                output half v; returns the staged bf16 (128, HALF_F) tile."""
                ysb = ys_pool.tile([128, 512], bf16, tag="ysb", name="ysb")
                nc.vector.tensor_copy(ysb, psum_hv)
                psum_t = psumt_pool.tile(
                    [128, HALF_F], mybir.dt.float32, tag="pt", name="psum_t"
                )
                for tt_ in range(4):
                    nc.tensor.matmul(
                        psum_t[:, ts(tt_, BATCH)],
                        ysb[:, ts(tt_, 128)],
                        s_sb,
                        start=True,
                        stop=True,
                    )
                hs = ts(v, HALF_F)
                z_t = chain.tile([128, HALF_F], mybir.dt.float32,
                                 tag="z", name="z_t")
                nc.vector.tensor_tensor(
                    z_t, psum_t, xb_sb[:, hs], mybir.AluOpType.add
                )
                stage_v = stage_pool.tile(
                    [128, HALF_F], bf16, tag=f"st{v}", name=f"stage{v}"
                )
                activation(
                    z_t,
                    stage_v,
                    also_f32=None if out_f32 is None else out_f32[:, hs],
                    width=HALF_F,
                )
                return stage_v

            def broadcast(stage_a, stage_b):
                """AllGather both staged halves into x_sb."""
                ag_in = dram.tile([128, CHUNK_F], bf16, tag="agi", name="ag_in")
                nc.sync.dma_start(out=ag_in[:, 0:HALF_F], in_=stage_a)
                nc.scalar.dma_start(out=ag_in[:, HALF_F:CHUNK_F], in_=stage_b)
                ag_out = dram.tile(
                    [128 * N_CORES, CHUNK_F], bf16, addr_space="Shared",
                    tag="ago", name="ag_out",
                )
                nc.gpsimd.collective_compute(
                    "AllGather",
                    mybir.AluOpType.bypass,
                    replica_groups=[list(range(N_CORES))],
                    ins=[ag_in.opt()],
                    outs=[ag_out.opt()],
                )
                # per-source-core chunk DMAs (two HWDGE engines) so the next
                # step's first quads start before the whole state has landed
                for c in range(N_CORES):
                    eng = nc.sync if c % 2 == 0 else nc.scalar
                    eng.dma_start(
                        out=x_sb[:, CHUNK_F * c : CHUNK_F * (c + 1)],
                        in_=ag_out[128 * c : 128 * (c + 1), :],
                    )

            # PE warm-keeping: DVE scratch copies act as coarse timers that
            # pace small dummy-matmul bursts through the AllGather window so
            # HAM never sees a >3.4us idle gap on the PE array.
            pace_cols = int(os.environ.get("PACE_COLS", "4096"))
            n_bursts = int(os.environ.get("WARM_BURSTS", "0"))
            warm_per = int(os.environ.get("WARM_PER", "30"))
            pw_a = pw_b = None
            if n_bursts > 0:
                pw_a = persist.tile([128, pace_cols], f32, name="pw_a")
                pw_b = persist.tile([128, pace_cols], f32, name="pw_b")
                nc.vector.memset(pw_a, 0.0)
                nc.vector.memset(pw_b, 0.0)

            def pe_warm():
                psum_w = psumt_pool.tile(
                    [128, 512], mybir.dt.float32, tag="pw", name="psum_w",
                    bufs=1,
                )

                def burst(dep):
                    for _ in range(warm_per):
                        wmm = nc.tensor.matmul(
                            psum_w[0:BATCH, :], s_sb, wt_sb[:, 0, 0:512],
                            start=True, stop=True,
                        )
                        if dep is not None:
                            bass._add_dep_helper(
                                wmm.ins, dep.ins, True, "pace warm mm"
                            )

                burst(None)
                for i in range(n_bursts):
                    src, dst = (pw_a, pw_b) if i % 2 == 0 else (pw_b, pw_a)
                    cp = nc.vector.tensor_copy(dst, src)
                    burst(cp)

            # ---- step 1: X1 = mml(X_bias) ------------------------------
            stage_halves = []
            for v in range(2):
                stage_v = stage_pool.tile(
                    [128, HALF_F], bf16, tag=f"st{v}", name=f"stage{v}"
                )
                activation(xb_sb[:, ts(v, HALF_F)], stage_v, width=HALF_F)
                stage_halves.append(stage_v)
            broadcast(*stage_halves)

            # ---- steps 2..50: X <- mml(W @ X + X_bias) -----------------
            n_quads = K_TILES // 4  # 16
            for step in range(MAX_STEPS - 1):
                last = step == MAX_STEPS - 2
                out_f32 = None
                if last:
                    out_f32 = stage_pool.tile(
                        [128, CHUNK_F], mybir.dt.float32, tag="of", name="out_f32"
                    )
                # main matmul, h (output half) outer so half 0's full tail
                # overlaps half 1's matmuls; 4-way column-tiled over K
                psum_h = [
                    psum_pool.tile(
                        [128, 512], mybir.dt.float32, tag="pa", name="psum_a"
                    ),
                    psum_pool.tile(
                        [128, 512], mybir.dt.float32, tag="pb", name="psum_b"
                    ),
                ]

                def mm_quads(h, quads):
                    for q in quads:
                        for j in range(4):
                            k = 4 * q + j
                            nc.tensor.matmul(
                                psum_h[h][32 * j : 32 * (j + 1), :],
                                x_sb[:, ts(k, BATCH)],
                                wt_sb[:, k, ts(h, 512)],
                                start=(q == 0),
                                stop=(q == n_quads - 1),
                                tile_position=(0, 32 * j),
                            )

                mm_quads(0, range(n_quads))
                mm_quads(1, range(n_quads // 2))
                stage_a = tail_half(psum_h[0], 0, out_f32)  # S-pass lands here
                mm_quads(1, range(n_quads // 2, n_quads))
                stage_b = tail_half(psum_h[1], 1, out_f32)
                if last:
                    nc.sync.dma_start(out=out[:], in_=out_f32)
                else:
                    broadcast(stage_a, stage_b)
                    pe_warm()

    nc.compile()
    return nc


def _prepare_in_maps(X_full, weights, bias, edge_mask):
    W = np.where(edge_mask, weights, 0.0).astype(np.float32)
    Xb = X_full.astype(np.float32).T + bias.astype(np.float32)  # (n, B)
    S = np.zeros((128, BATCH), np.float32)
    S[np.arange(128), np.arange(128) % BATCH] = 1.0
    S = S.astype(ml_dtypes.bfloat16)
    in_maps = []
    for c in range(N_CORES):
        rows = slice(LOCAL * c, LOCAL * (c + 1))
        wt_c = np.ascontiguousarray(W[rows, :].T).astype(ml_dtypes.bfloat16)
        xb_c = (
            Xb[rows]                       # (1024, 32)
            .reshape(LOCAL_TILES, 128, BATCH)
            .transpose(1, 0, 2)
            .reshape(128, CHUNK_F)
            .copy()
        )
        in_maps.append({"wt": wt_c, "xb": xb_c, "s_in": S})
    return in_maps


def _reassemble(results):
    out = np.empty((BATCH, N_NODES), np.float32)
    for c in range(N_CORES):
        oc = np.asarray(results[c]["out"])  # (128, 256)
        chunk = (
            oc.reshape(128, LOCAL_TILES, BATCH)
            .transpose(1, 0, 2)
            .reshape(LOCAL, BATCH)
        )
        out[:, LOCAL * c : LOCAL * (c + 1)] = chunk.T
    return out


def kernel(X_full, weights, bias, edge_mask):
    global LAST_RESULTS
    setup_tracing()
    in_maps = _prepare_in_maps(X_full, weights, bias, edge_mask)
    nc = build_nc()
    res = run_bass_kernel_spmd(nc, in_maps, core_ids=list(range(N_CORES)))
    LAST_RESULTS = res
    return _reassemble(res.results)


if __name__ == "__main__":
    # quick self-run with random data
    rng = np.random.default_rng(0)
    X_full = rng.random((BATCH, N_NODES), np.float32)
    weights = rng.standard_normal((N_NODES, N_NODES), np.float32)
    bias = 0.001 * np.ones((N_NODES, 1), np.float32)
    edge_mask = rng.random((N_NODES, N_NODES)) < 0.002
    out = kernel(X_full, weights, bias, edge_mask)
    print("out", out.shape, out.dtype, out[:2, :4])
